# revision 68
# baseline (speedup 1.0000x reference)
"""Coattention kernel for Trainium2, data-parallel over batch across 8 NeuronCores.

v4 design (per core, one batch).  Decomposed score S = u_i + v_j + S0_ij with
S0 = (C*w3)@Qp^T; the u/v terms are folded so that every fp8 tensor is
O(1)-ranged (a per-row scale that cancels mathematically does NOT cancel in
fp8 -- it crushes rows into subnormals):

  Qp^T    = Wq-GEMM (fp16); Qp natural via PE transpose
  E1NOU   = exp(S0 - SHIFT)            fp16, no u/v (Act, const bias)
  e18     = fp8(E1NOU * e^u)           natural [c,q]; T-path lhsT (u belongs
                                       to the column softmax)
  T path  = fp8 DoubleRow: T = e18^T @ C_8, cs = e18^T @ 1; tv = T/cs
  e1t(8)  = transpose(E1NOU) * e^v     [q,c] fp16 + fp8 (v is per-partition
                                       after transposing; row softmax weights)
  A       = (e1t^T @ Qp) / r16,  r16 = e1t^T @ [1|0]      (fp16)
  Bm      = (e1t8^T @ tv8) / r8, r8  = e1t8^T @ 1         (fp8 DoubleRow)
  out     = [A | C*A | C*Bm] staged fp16, f32 upcast + C passthrough on host

Scheduling: PE warmup ramps the p-state during the first DMAs; qt/wq ship
interleaved per k-chunk and ct in column chunks so P1/S0 chase the (single,
serialized) DMA wire; S0+exp pipeline per 4-ic block with h0 E1-transposes
mid-stream; T right after the last exp; h1 transposes and fp8 copies spread
through the first A iterations; per-ic outputs stream as singles on the SP
queue (gpsimd must never touch PSUM; all output DMAs live on sync).

Masks are all-ones and b == bq == 0 per the problem spec, so they drop out.
"""

import os
import sys

import numpy as np
import ml_dtypes

for _p in ("/opt/trn_rl_repo", "/root/.axon_site/_ro/trn_rl_repo"):
    if os.path.isdir(_p) and _p not in sys.path:
        sys.path.append(_p)

import concourse.bass as bass
import concourse.mybir as mybir
import concourse.tile as tile
from concourse.bass_utils import run_bass_kernel_spmd

C_LEN, Q_LEN, DIM, B = 2048, 512, 512, 8
N_CORES = 8
IC = C_LEN // 128   # 16 i-chunks
JC = Q_LEN // 128   # 4 j-chunks
KT = DIM // 128     # 4 k-tiles
SHIFT = 3.5         # global logit shift so fp8 E1 stays in e4m3 range

F32 = mybir.dt.float32
F16 = mybir.dt.float16
F8 = mybir.dt.float8e4
EXP = mybir.ActivationFunctionType.Exp
DR = mybir.MatmulPerfMode.DoubleRow
MUL = mybir.AluOpType.mult
NPF8 = ml_dtypes.float8_e4m3

BM_FP8 = True  # Bm matmul via fp8 DoubleRow (needs E1T8 copies)


def _split_multi_waits(nc, cap=1):
    """Walrus in this container rejects >1 sync wait per CTRL instruction;
    Tile's tail drain carries one wait per tracked processor.  Spill the
    extras onto preceding single-wait NoOps on the same engine."""
    for fn in nc.m.functions:
        for blk in fn.blocks:
            insts = list(blk.instructions)
            out, changed = [], False
            for inst in insts:
                si = inst.sync_info
                ow = si.on_wait if si is not None else None
                if ow is not None and len(ow) > cap:
                    waits = list(ow)
                    for w in waits[:-cap]:
                        nop = mybir.InstNoOp(
                            name=nc.get_next_instruction_name(), ins=[], outs=[]
                        )
                        nop.engine = inst.engine
                        nop.sync_info = mybir.SyncInfo(on_wait=[w], on_update=[])
                        out.append(nop)
                    si.on_wait = waits[-cap:]
                    changed = True
                out.append(inst)
            if changed:
                blk.instructions = out


def _build_program(split_waits=True):
    nc = bass.Bass()

    QW = nc.dram_tensor("QW", [DIM, Q_LEN + DIM], F16, kind="ExternalInput")
    CT = nc.dram_tensor("CT", [DIM, C_LEN], F16, kind="ExternalInput")
    CN = nc.dram_tensor("CN", [C_LEN, DIM], F16, kind="ExternalInput")
    CN8 = nc.dram_tensor("CN8", [C_LEN, DIM], F8, kind="ExternalInput")
    CF32 = nc.dram_tensor("CF32", [128, KT + 1], F32, kind="ExternalInput")
    CF16 = nc.dram_tensor("CF16", [128, 8 + 2 + 128], F16, kind="ExternalInput")
    CF8 = nc.dram_tensor("CF8", [128, 4], F8, kind="ExternalInput")
    Y = nc.dram_tensor("Y", [C_LEN, 3 * DIM], F16, kind="ExternalOutput")

    with tile.TileContext(nc) as tc:
        with (
            tc.tile_pool(name="consts", bufs=1) as consts,
            tc.tile_pool(name="big", bufs=1) as big,
            tc.tile_pool(name="ps_mm", bufs=5, space="PSUM") as ps_mm,
            tc.tile_pool(name="ps_t", bufs=2, space="PSUM") as ps_t,
            tc.tile_pool(name="ps_vec", bufs=1, space="PSUM") as ps_vec,
            tc.tile_pool(name="scr", bufs=3) as scr,
            tc.tile_pool(name="stage", bufs=8) as stagep,
        ):
            # ---- SBUF tiles ----
            qw_sb = big.tile([128, KT, Q_LEN + DIM], F16)  # [Q^T | Wq] per kt
            ct_sb = big.tile([128, KT, C_LEN], F16)
            cn_sb = big.tile([128, IC, DIM], F16)
            cn8_sb = big.tile([128, IC, DIM], F8)
            qpt_sb = big.tile([128, KT, Q_LEN], F16)    # Qp^T plain
            qptw3_sb = big.tile([128, KT, Q_LEN], F16)  # w3 * Qp^T
            qp_sb = big.tile([128, JC, DIM], F16)       # Qp natural
            tv_sb = big.tile([128, JC, DIM], F8 if BM_FP8 else F16)  # T/cs
            e1_sb = big.tile([128, IC, Q_LEN], F16)     # exp(S0 + u - SHIFT)
            e18_sb = big.tile([128, IC, Q_LEN], F8)     # fp8 copy
            e1t_sb = big.tile([128, JC, C_LEN], F16)    # E1 transposed
            if BM_FP8:
                e1t8_sb = big.tile([128, JC, C_LEN], F8)

            cf32 = consts.tile([128, KT + 1], F32)      # w3 columns | -SHIFT
            cf16 = consts.tile([128, 8 + 2 + 128], F16)
            cf8 = consts.tile([128, 2, 2], F8)          # [:, :, 0] = 1
            w1r = cf16[:, 0:KT]
            w2r = cf16[:, KT : 2 * KT]
            ones16 = cf16[:, 8:10]                      # [1 | 0]
            idt = cf16[:, 10 : 10 + 128]
            shiftcol = cf32[:, KT : KT + 1]
            ev_sb = consts.tile([128, JC], F32)
            eu_sb = consts.tile([128, IC], F32)

            # ---- PE warmup: junk matmuls on a memset tile ramp the p-state
            # while the first input DMAs are in flight.  They write the P1
            # psum tiles (start=True of the real P1 groups resets them, and
            # the tiles do get read, which keeps the BIR verifier happy). ----
            warm = consts.tile([128, 512], F16)
            nc.gpsimd.memset(warm, 0.5)
            qp_ps = [
                ps_mm.tile([128, Q_LEN], F32, tag="mm", name=f"qp_ps{m}")
                for m in range(KT)
            ]
            for i in range(6):
                nc.tensor.matmul(
                    qp_ps[i % KT], warm[:, 0:128], warm, start=True, stop=True
                )

            # ---- input DMAs (order = wire order; one serialized channel) ----
            for kt in range(KT):
                nc.sync.dma_start(
                    out=qw_sb[:, kt, :], in_=QW[kt * 128 : (kt + 1) * 128, :]
                )
            # ct in COLUMN chunks matching the S0 ic-blocks, so u/S0/exp for
            # each block only wait on their own chunk
            S0_BLOCKS = [(0, 4), (4, 4), (8, 4), (12, 2), (14, 2)]

            def ct_chunk(b0, bn):
                nc.sync.dma_start(
                    out=ct_sb[:, :, b0 * 128 : (b0 + bn) * 128],
                    in_=CT[:, b0 * 128 : (b0 + bn) * 128].rearrange(
                        "(k p) c -> p k c", p=128
                    ),
                )

            ct_chunk(*S0_BLOCKS[0])
            nc.sync.dma_start(out=cf16, in_=CF16[:])
            nc.sync.dma_start(out=cf32, in_=CF32[:])
            nc.sync.dma_start(
                out=cf8, in_=CF8[:].rearrange("p (a b) -> p a b", a=2)
            )
            for b0, bn in S0_BLOCKS[1:]:
                ct_chunk(b0, bn)
            for h in range(2):
                nc.sync.dma_start(
                    out=cn8_sb[:, 8 * h : 8 * h + 8, :],
                    in_=CN8[h * 1024 : (h + 1) * 1024, :].rearrange(
                        "(ic p) e -> p ic e", p=128
                    ),
                )
            for h in range(2):
                nc.sync.dma_start(
                    out=cn_sb[:, 8 * h : 8 * h + 8, :],
                    in_=CN[h * 1024 : (h + 1) * 1024, :].rearrange(
                        "(ic p) e -> p ic e", p=128
                    ),
                )

            # ---- P1: Qp^T GEMM, kt-outer to chase the qt/wq DMAs ----
            for kt in range(KT):
                for m in range(KT):
                    nc.tensor.matmul(
                        qp_ps[m],
                        qw_sb[:, kt, Q_LEN + m * 128 : Q_LEN + (m + 1) * 128],
                        qw_sb[:, kt, 0:Q_LEN],
                        start=(kt == 0),
                        stop=(kt == KT - 1),
                    )
            for m in range(KT):
                nc.scalar.copy(qpt_sb[:, m, :], qp_ps[m])
                nc.vector.tensor_scalar_mul(
                    qptw3_sb[:, m, :], qp_ps[m], cf32[:, m : m + 1]
                )

            # ---- v = Qp @ w2, ev, Qp-natural transposes, qv: deferred until
            # after S0 block 0 (they need the Act-side qpt copies; S0 only
            # needs the DVE-side qptw3, so it can start sooner) ----
            def v_qv_phase():
                v_ps = ps_vec.tile([128, JC], F32, tag="vec")
                for jc in range(JC):
                    for kt in range(KT):
                        nc.tensor.matmul(
                            v_ps[:, jc : jc + 1],
                            qpt_sb[:, kt, jc * 128 : (jc + 1) * 128],
                            w2r[:, kt : kt + 1],
                            start=(kt == 0),
                            stop=(kt == KT - 1),
                        )
                nc.scalar.activation(out=ev_sb, in_=v_ps, func=EXP)
                for jc in range(JC):
                    tpq = ps_t.tile([128, DIM], F16, tag="tp", name=f"tpq{jc}")
                    for m in range(KT):
                        nc.tensor.matmul(
                            tpq[:, m * 128 : (m + 1) * 128],
                            qpt_sb[:, m, jc * 128 : (jc + 1) * 128],
                            idt,
                            is_transpose=True,
                        )
                    nc.vector.tensor_copy(qp_sb[:, jc, :], tpq)

            # ---- S0 + u + exp pipelined per ic-block; each block only needs
            # its own ct column chunk (tapered blocks: last exps finish early)
            def s0_u_exp_block(b0, bn):
                for i4 in range(bn):
                    ic = b0 + i4
                    for kt in range(KT):
                        nc.tensor.matmul(
                            u_ps[:, ic : ic + 1],
                            ct_sb[:, kt, ic * 128 : (ic + 1) * 128],
                            w1r[:, kt : kt + 1],
                            start=(kt == 0),
                            stop=(kt == KT - 1),
                        )
                nc.scalar.activation(
                    out=eu_sb[:, b0 : b0 + bn], in_=u_ps[:, b0 : b0 + bn],
                    func=EXP,
                )
                pss = [
                    ps_mm.tile([128, Q_LEN], F32, tag="mm", name=f"s0_ps{b0}_{i}")
                    for i in range(bn)
                ]
                for i4 in range(bn):
                    ic = b0 + i4
                    for kt in range(KT):
                        nc.tensor.matmul(
                            pss[i4],
                            ct_sb[:, kt, ic * 128 : (ic + 1) * 128],
                            qptw3_sb[:, kt, :],
                            start=(kt == 0),
                            stop=(kt == KT - 1),
                        )
                for i4 in range(bn):
                    ic = b0 + i4
                    nc.scalar.activation(
                        out=e1_sb[:, ic, :], in_=pss[i4], func=EXP,
                        bias=shiftcol,
                    )
                    nc.gpsimd.tensor_scalar_mul(
                        e18_sb[:, ic, :], e1_sb[:, ic, :], eu_sb[:, ic : ic + 1]
                    )

            # ---- E1T transposes (half-tiles) interleaved with fp8-DR T/cs.
            # h0 halves (ic 0-7 columns) come first so the A-phase can start
            # while the h1 halves are still transposing. ----
            def tp_half(jc, hf):
                jcb = slice(jc * 128, (jc + 1) * 128)
                tph = ps_t.tile(
                    [128, C_LEN // 2], F16, tag="tp", name=f"tp{jc}_{hf}"
                )
                for i8 in range(8):
                    ic = hf * 8 + i8
                    nc.tensor.matmul(
                        tph[:, i8 * 128 : (i8 + 1) * 128],
                        e1_sb[:, ic, jcb],
                        idt,
                        is_transpose=True,
                    )
                hsl = slice(hf * 1024, (hf + 1) * 1024)
                # fold e^v into the transposed weights (per-partition here)
                nc.vector.tensor_scalar_mul(
                    e1t_sb[:, jc, hsl], tph, ev_sb[:, jc : jc + 1]
                )
                if BM_FP8 and hf == 0:
                    # h0 fp8 copies run mid-S0 where DVE has slack; the h1
                    # fp8 copies are issued in 512-wide pieces from the SBUF
                    # e1t16 during the first A iterations (Act)
                    nc.vector.tensor_scalar_mul(
                        e1t8_sb[:, jc, hsl], tph, ev_sb[:, jc : jc + 1]
                    )

            def t_phase(jc):
                jcb = slice(jc * 128, (jc + 1) * 128)
                t_ps = ps_mm.tile([128, DIM], F32, tag="mm")
                cs_ps = ps_vec.tile([128, 2], F32, tag="vec")
                for h in range(IC // 2):
                    lhsT = e18_sb[:, 2 * h : 2 * h + 2, jcb]
                    nc.tensor.matmul(
                        t_ps, lhsT, cn8_sb[:, 2 * h : 2 * h + 2, :],
                        start=(h == 0), stop=(h == IC // 2 - 1),
                        perf_mode=DR,
                    )
                    nc.tensor.matmul(
                        cs_ps[:, 0:1], lhsT, cf8[:, :, 0:1],
                        start=(h == 0), stop=(h == IC // 2 - 1),
                        perf_mode=DR,
                    )
                rcs = scr.tile([128, 1], F32, tag="rr")
                nc.vector.reciprocal(out=rcs, in_=cs_ps[:, 0:1])
                nc.vector.tensor_scalar_mul(tv_sb[:, jc, :], t_ps, rcs)

            u_ps = ps_vec.tile([128, IC], F32, tag="vec")
            v_qv_phase()
            s0_u_exp_block(0, 4)
            s0_u_exp_block(4, 4)
            # h0 transposes only need e1 rows ic0-7: run them mid-S0
            tp_half(0, 0)
            tp_half(1, 0)
            s0_u_exp_block(8, 4)
            tp_half(2, 0)
            tp_half(3, 0)
            s0_u_exp_block(12, 2)
            s0_u_exp_block(14, 2)
            # h1 transposes in 4-wide chunks: jc0/jc1 interleave with T,
            # jc2/jc3 spread over the first A iterations so per-ic PE work
            # stays under the output wire pace
            def tp_chunk(c):
                jcq, hh = divmod(c, 2)
                jcb = slice(jcq * 128, (jcq + 1) * 128)
                tpc = ps_t.tile([128, 512], F16, tag="tp", name=f"tpc{c}")
                for i4 in range(4):
                    nc.tensor.matmul(
                        tpc[:, i4 * 128 : (i4 + 1) * 128],
                        e1_sb[:, 8 + 4 * hh + i4, jcb],
                        idt,
                        is_transpose=True,
                    )
                seg = slice(1024 + hh * 512, 1024 + (hh + 1) * 512)
                nc.vector.tensor_scalar_mul(
                    e1t_sb[:, jcq, seg], tpc, ev_sb[:, jcq : jcq + 1]
                )
                if BM_FP8:
                    nc.scalar.mul(
                        e1t8_sb[:, jcq, seg], tpc, ev_sb[:, jcq : jcq + 1]
                    )

            t_phase(0)
            tp_chunk(0)
            tp_chunk(1)
            t_phase(1)
            tp_chunk(2)
            tp_chunk(3)
            t_phase(2)
            t_phase(3)

            # ---- A, Bm, r per ic; stage [A | C*A | C*Bm].  The h1 E1T
            # transposes are interleaved with the first ics (which only read
            # h0 columns).  Paired output DMAs, singles for the last 4. ----
            st = None
            r_tile = ps_vec.tile([128, 2, 4], F32, tag="vec")
            for ic in range(IC):
                if ic < 4:
                    tp_chunk(4 + ic)
                icb = slice(ic * 128, (ic + 1) * 128)
                a_ps = ps_mm.tile([128, DIM], F32, tag="mm")
                b_ps = ps_mm.tile([128, DIM], F32, tag="mm")
                r_ps = r_tile[:, ic % 2, :]
                for jc in range(JC):
                    lhsT = e1t_sb[:, jc, icb]
                    nc.tensor.matmul(
                        a_ps, lhsT, qp_sb[:, jc, :],
                        start=(jc == 0), stop=(jc == JC - 1),
                    )
                    nc.tensor.matmul(
                        r_ps[:, 0:2], lhsT, ones16,
                        start=(jc == 0), stop=(jc == JC - 1),
                    )
                if BM_FP8:
                    for h in range(JC // 2):
                        nc.tensor.matmul(
                            b_ps,
                            e1t8_sb[:, 2 * h : 2 * h + 2, icb],
                            tv_sb[:, 2 * h : 2 * h + 2, :],
                            start=(h == 0), stop=(h == JC // 2 - 1),
                            perf_mode=DR,
                        )
                        nc.tensor.matmul(
                            r_ps[:, 2:3],
                            e1t8_sb[:, 2 * h : 2 * h + 2, icb],
                            cf8[:, :, 0:1],
                            start=(h == 0), stop=(h == JC // 2 - 1),
                            perf_mode=DR,
                        )
                else:
                    for jc in range(JC):
                        nc.tensor.matmul(
                            b_ps, e1t_sb[:, jc, icb], tv_sb[:, jc, :],
                            start=(jc == 0), stop=(jc == JC - 1),
                        )
                rr = scr.tile([128, 1], F32, tag="rr")
                nc.vector.reciprocal(out=rr, in_=r_ps[:, 0:1])
                if BM_FP8:
                    rr8 = scr.tile([128, 1], F32, tag="rr")
                    nc.vector.reciprocal(out=rr8, in_=r_ps[:, 2:3])
                else:
                    rr8 = rr
                st = stagep.tile(
                    [128, 1, 3 * DIM], F16, tag="stage", name=f"st{ic}"
                )
                sl = st[:, 0, :]
                nc.scalar.mul(sl[:, 0:DIM], a_ps, rr)
                # gpsimd cannot touch PSUM: Pool multiplies the SBUF st_A
                # copy by C; DVE handles C*Bm straight from the psum
                nc.gpsimd.tensor_mul(
                    sl[:, DIM : 2 * DIM], sl[:, 0:DIM], cn_sb[:, ic, :]
                )
                nc.vector.scalar_tensor_tensor(
                    out=sl[:, 2 * DIM : 3 * DIM], in0=b_ps, scalar=rr8,
                    in1=cn_sb[:, ic, :], op0=MUL, op1=MUL,
                )
                nc.sync.dma_start(
                    out=Y[icb, :], in_=st.rearrange("p a e -> p (a e)")
                )

    if split_waits:
        _split_multi_waits(nc)
    return nc


_PROGRAM = None


def _get_program():
    global _PROGRAM
    if _PROGRAM is None:
        _PROGRAM = _build_program()
    return _PROGRAM


def kernel(C, Q, C_mask, Q_mask, Wq, bq, w1, w2, w3, b):
    # Masks are all-ones and bq/b are zero for this problem (spec fills);
    # they cancel out of the computation and are not shipped to the device.
    C = np.asarray(C, np.float32)
    Q = np.asarray(Q, np.float32)
    Wq = np.asarray(Wq, np.float32)
    w1 = np.asarray(w1, np.float32)
    w2 = np.asarray(w2, np.float32)
    w3 = np.asarray(w3, np.float32)

    cf32 = np.zeros((128, KT + 1), np.float32)
    cf32[:, 0:KT] = w3.reshape(KT, 128).T
    cf32[:, KT] = -SHIFT
    cf16 = np.zeros((128, 8 + 2 + 128), np.float16)
    cf16[:, 0:KT] = w1.reshape(KT, 128).T
    cf16[:, KT : 2 * KT] = w2.reshape(KT, 128).T
    cf16[:, 8] = 1.0
    cf16[:, 10 : 10 + 128] = np.eye(128, dtype=np.float16)
    cf8 = np.zeros((128, 4), NPF8)
    cf8[:, 0] = 1.0
    cf8[:, 2] = 1.0
    wq16 = Wq.astype(np.float16)

    in_maps = []
    for bi in range(B):
        Cb = np.ascontiguousarray(C[:, bi, :])
        Cb16 = Cb.astype(np.float16)
        qw = np.concatenate(
            [Q[:, bi, :].T.astype(np.float16), wq16], axis=1
        )
        in_maps.append(
            {
                "QW": np.ascontiguousarray(qw),
                "CT": np.ascontiguousarray(Cb.T).astype(np.float16),
                "CN": Cb16,
                "CN8": Cb.astype(NPF8),
                "CF32": cf32,
                "CF16": cf16,
                "CF8": cf8,
            }
        )

    nc = _get_program()
    res = run_bass_kernel_spmd(nc, in_maps, list(range(N_CORES)))
    out = np.empty((C_LEN, B, 4 * DIM), np.float32)
    out[:, :, 0:DIM] = C  # passthrough columns assembled on host
    for c in range(N_CORES):
        out[:, c, DIM:] = res.results[c]["Y"].astype(np.float32)
    return out


# revision 71
# speedup vs baseline: 1.0007x; 1.0007x over previous
"""Coattention kernel for Trainium2, data-parallel over batch across 8 NeuronCores.

v4 design (per core, one batch).  Decomposed score S = u_i + v_j + S0_ij with
S0 = (C*w3)@Qp^T; the u/v terms are folded so that every fp8 tensor is
O(1)-ranged (a per-row scale that cancels mathematically does NOT cancel in
fp8 -- it crushes rows into subnormals):

  Qp^T    = Wq-GEMM (fp16); Qp natural via PE transpose
  E1NOU   = exp(S0 - SHIFT)            fp16, no u/v (Act, const bias)
  e18     = fp8(E1NOU * e^u)           natural [c,q]; T-path lhsT (u belongs
                                       to the column softmax)
  T path  = fp8 DoubleRow: T = e18^T @ C_8, cs = e18^T @ 1; tv = T/cs
  e1t(8)  = transpose(E1NOU) * e^v     [q,c] fp16 + fp8 (v is per-partition
                                       after transposing; row softmax weights)
  A       = (e1t^T @ Qp) / r16,  r16 = e1t^T @ [1|0]      (fp16)
  Bm      = (e1t8^T @ tv8) / r8, r8  = e1t8^T @ 1         (fp8 DoubleRow)
  out     = [A | C*A | C*Bm] staged fp16, f32 upcast + C passthrough on host

Scheduling: PE warmup ramps the p-state during the first DMAs; qt/wq ship
interleaved per k-chunk and ct in column chunks so P1/S0 chase the (single,
serialized) DMA wire; S0+exp pipeline per 4-ic block with h0 E1-transposes
mid-stream; T right after the last exp; h1 transposes and fp8 copies spread
through the first A iterations; per-ic outputs stream as singles on the SP
queue (gpsimd must never touch PSUM; all output DMAs live on sync).

Masks are all-ones and b == bq == 0 per the problem spec, so they drop out.
"""

import os
import sys

import numpy as np
import ml_dtypes

for _p in ("/opt/trn_rl_repo", "/root/.axon_site/_ro/trn_rl_repo"):
    if os.path.isdir(_p) and _p not in sys.path:
        sys.path.append(_p)

import concourse.bass as bass
import concourse.mybir as mybir
import concourse.tile as tile
from concourse.bass_utils import run_bass_kernel_spmd

C_LEN, Q_LEN, DIM, B = 2048, 512, 512, 8
N_CORES = 8
IC = C_LEN // 128   # 16 i-chunks
JC = Q_LEN // 128   # 4 j-chunks
KT = DIM // 128     # 4 k-tiles
SHIFT = 3.5         # global logit shift so fp8 E1 stays in e4m3 range

F32 = mybir.dt.float32
F16 = mybir.dt.float16
F8 = mybir.dt.float8e4
EXP = mybir.ActivationFunctionType.Exp
DR = mybir.MatmulPerfMode.DoubleRow
MUL = mybir.AluOpType.mult
NPF8 = ml_dtypes.float8_e4m3

BM_FP8 = True  # Bm matmul via fp8 DoubleRow (needs E1T8 copies)


def _split_multi_waits(nc, cap=1):
    """Walrus in this container rejects >1 sync wait per CTRL instruction;
    Tile's tail drain carries one wait per tracked processor.  Spill the
    extras onto preceding single-wait NoOps on the same engine."""
    for fn in nc.m.functions:
        for blk in fn.blocks:
            insts = list(blk.instructions)
            out, changed = [], False
            for inst in insts:
                si = inst.sync_info
                ow = si.on_wait if si is not None else None
                if ow is not None and len(ow) > cap:
                    waits = list(ow)
                    for w in waits[:-cap]:
                        nop = mybir.InstNoOp(
                            name=nc.get_next_instruction_name(), ins=[], outs=[]
                        )
                        nop.engine = inst.engine
                        nop.sync_info = mybir.SyncInfo(on_wait=[w], on_update=[])
                        out.append(nop)
                    si.on_wait = waits[-cap:]
                    changed = True
                out.append(inst)
            if changed:
                blk.instructions = out


def _build_program(split_waits=True):
    nc = bass.Bass()

    QW = nc.dram_tensor("QW", [DIM, Q_LEN + DIM], F16, kind="ExternalInput")
    CT = nc.dram_tensor("CT", [DIM, C_LEN], F16, kind="ExternalInput")
    CN = nc.dram_tensor("CN", [C_LEN, DIM], F16, kind="ExternalInput")
    CN8 = nc.dram_tensor("CN8", [C_LEN, DIM], F8, kind="ExternalInput")
    CF32 = nc.dram_tensor("CF32", [128, KT + 1], F32, kind="ExternalInput")
    CF16 = nc.dram_tensor("CF16", [128, 8 + 2 + 128], F16, kind="ExternalInput")
    CF8 = nc.dram_tensor("CF8", [128, 4], F8, kind="ExternalInput")
    Y = nc.dram_tensor("Y", [C_LEN, 3 * DIM], F16, kind="ExternalOutput")

    with tile.TileContext(nc) as tc:
        with (
            tc.tile_pool(name="consts", bufs=1) as consts,
            tc.tile_pool(name="big", bufs=1) as big,
            tc.tile_pool(name="ps_mm", bufs=5, space="PSUM") as ps_mm,
            tc.tile_pool(name="ps_t", bufs=2, space="PSUM") as ps_t,
            tc.tile_pool(name="ps_vec", bufs=1, space="PSUM") as ps_vec,
            tc.tile_pool(name="scr", bufs=3) as scr,
            tc.tile_pool(name="stage", bufs=8) as stagep,
        ):
            # ---- SBUF tiles ----
            qw_sb = big.tile([128, KT, Q_LEN + DIM], F16)  # [Q^T | Wq] per kt
            ct_sb = big.tile([128, KT, C_LEN], F16)
            cn_sb = big.tile([128, IC, DIM], F16)
            cn8_sb = big.tile([128, IC, DIM], F8)
            qpt_sb = big.tile([128, KT, Q_LEN], F16)    # Qp^T plain
            qptw3_sb = big.tile([128, KT, Q_LEN], F16)  # w3 * Qp^T
            qp_sb = big.tile([128, JC, DIM], F16)       # Qp natural
            tv_sb = big.tile([128, JC, DIM], F8 if BM_FP8 else F16)  # T/cs
            e1_sb = big.tile([128, IC, Q_LEN], F16)     # exp(S0 + u - SHIFT)
            e18_sb = big.tile([128, IC, Q_LEN], F8)     # fp8 copy
            e1t_sb = big.tile([128, JC, C_LEN], F16)    # E1 transposed
            if BM_FP8:
                e1t8_sb = big.tile([128, JC, C_LEN], F8)

            cf32 = consts.tile([128, KT + 1], F32)      # w3 columns | -SHIFT
            cf16 = consts.tile([128, 8 + 2 + 128], F16)
            cf8 = consts.tile([128, 2, 2], F8)          # [:, :, 0] = 1
            w1r = cf16[:, 0:KT]
            w2r = cf16[:, KT : 2 * KT]
            ones16 = cf16[:, 8:10]                      # [1 | 0]
            idt = cf16[:, 10 : 10 + 128]
            shiftcol = cf32[:, KT : KT + 1]
            ev_sb = consts.tile([128, JC], F32)
            eu_sb = consts.tile([128, IC], F32)

            # ---- PE warmup: junk matmuls on a memset tile ramp the p-state
            # while the first input DMAs are in flight.  They write the P1
            # psum tiles (start=True of the real P1 groups resets them, and
            # the tiles do get read, which keeps the BIR verifier happy). ----
            warm = consts.tile([128, 512], F16)
            nc.gpsimd.memset(warm, 0.5)
            qp_ps = [
                ps_mm.tile([128, Q_LEN], F32, tag="mm", name=f"qp_ps{m}")
                for m in range(KT)
            ]
            for i in range(6):
                nc.tensor.matmul(
                    qp_ps[i % KT], warm[:, 0:128], warm, start=True, stop=True
                )

            # ---- input DMAs (order = wire order; one serialized channel) ----
            for kt in range(KT):
                nc.sync.dma_start(
                    out=qw_sb[:, kt, :], in_=QW[kt * 128 : (kt + 1) * 128, :]
                )
            # ct in COLUMN chunks matching the S0 ic-blocks, so u/S0/exp for
            # each block only wait on their own chunk
            S0_BLOCKS = [(0, 4), (4, 4), (8, 4), (12, 2), (14, 2)]

            def ct_chunk(b0, bn):
                nc.sync.dma_start(
                    out=ct_sb[:, :, b0 * 128 : (b0 + bn) * 128],
                    in_=CT[:, b0 * 128 : (b0 + bn) * 128].rearrange(
                        "(k p) c -> p k c", p=128
                    ),
                )

            ct_chunk(*S0_BLOCKS[0])
            nc.sync.dma_start(out=cf16, in_=CF16[:])
            nc.sync.dma_start(out=cf32, in_=CF32[:])
            nc.sync.dma_start(
                out=cf8, in_=CF8[:].rearrange("p (a b) -> p a b", a=2)
            )
            for b0, bn in S0_BLOCKS[1:]:
                ct_chunk(b0, bn)
            for h in range(2):
                nc.sync.dma_start(
                    out=cn8_sb[:, 8 * h : 8 * h + 8, :],
                    in_=CN8[h * 1024 : (h + 1) * 1024, :].rearrange(
                        "(ic p) e -> p ic e", p=128
                    ),
                )
            for h in range(2):
                nc.sync.dma_start(
                    out=cn_sb[:, 8 * h : 8 * h + 8, :],
                    in_=CN[h * 1024 : (h + 1) * 1024, :].rearrange(
                        "(ic p) e -> p ic e", p=128
                    ),
                )

            # ---- P1: Qp^T GEMM, kt-outer to chase the qt/wq DMAs ----
            for kt in range(KT):
                for m in range(KT):
                    nc.tensor.matmul(
                        qp_ps[m],
                        qw_sb[:, kt, Q_LEN + m * 128 : Q_LEN + (m + 1) * 128],
                        qw_sb[:, kt, 0:Q_LEN],
                        start=(kt == 0),
                        stop=(kt == KT - 1),
                    )
            for m in range(KT):
                nc.scalar.copy(qpt_sb[:, m, :], qp_ps[m])
                nc.vector.tensor_scalar_mul(
                    qptw3_sb[:, m, :], qp_ps[m], cf32[:, m : m + 1]
                )

            # ---- v = Qp @ w2, ev, Qp-natural transposes, qv: deferred until
            # after S0 block 0 (they need the Act-side qpt copies; S0 only
            # needs the DVE-side qptw3, so it can start sooner) ----
            def v_qv_phase():
                v_ps = ps_vec.tile([128, JC], F32, tag="vec")
                for jc in range(JC):
                    for kt in range(KT):
                        nc.tensor.matmul(
                            v_ps[:, jc : jc + 1],
                            qpt_sb[:, kt, jc * 128 : (jc + 1) * 128],
                            w2r[:, kt : kt + 1],
                            start=(kt == 0),
                            stop=(kt == KT - 1),
                        )
                nc.scalar.activation(out=ev_sb, in_=v_ps, func=EXP)
                for jc in range(JC):
                    tpq = ps_t.tile([128, DIM], F16, tag="tp", name=f"tpq{jc}")
                    for m in range(KT):
                        nc.tensor.matmul(
                            tpq[:, m * 128 : (m + 1) * 128],
                            qpt_sb[:, m, jc * 128 : (jc + 1) * 128],
                            idt,
                            is_transpose=True,
                        )
                    nc.vector.tensor_copy(qp_sb[:, jc, :], tpq)

            # ---- S0 + u + exp pipelined per ic-block; each block only needs
            # its own ct column chunk (tapered blocks: last exps finish early)
            def s0_u_exp_block(b0, bn):
                for i4 in range(bn):
                    ic = b0 + i4
                    for kt in range(KT):
                        nc.tensor.matmul(
                            u_ps[:, ic : ic + 1],
                            ct_sb[:, kt, ic * 128 : (ic + 1) * 128],
                            w1r[:, kt : kt + 1],
                            start=(kt == 0),
                            stop=(kt == KT - 1),
                        )
                nc.scalar.activation(
                    out=eu_sb[:, b0 : b0 + bn], in_=u_ps[:, b0 : b0 + bn],
                    func=EXP,
                )
                pss = [
                    ps_mm.tile([128, Q_LEN], F32, tag="mm", name=f"s0_ps{b0}_{i}")
                    for i in range(bn)
                ]
                for i4 in range(bn):
                    ic = b0 + i4
                    for kt in range(KT):
                        nc.tensor.matmul(
                            pss[i4],
                            ct_sb[:, kt, ic * 128 : (ic + 1) * 128],
                            qptw3_sb[:, kt, :],
                            start=(kt == 0),
                            stop=(kt == KT - 1),
                        )
                for i4 in range(bn):
                    ic = b0 + i4
                    nc.scalar.activation(
                        out=e1_sb[:, ic, :], in_=pss[i4], func=EXP,
                        bias=shiftcol,
                    )
                    nc.gpsimd.tensor_scalar_mul(
                        e18_sb[:, ic, :], e1_sb[:, ic, :], eu_sb[:, ic : ic + 1]
                    )

            # ---- E1T transposes (half-tiles) interleaved with fp8-DR T/cs.
            # h0 halves (ic 0-7 columns) come first so the A-phase can start
            # while the h1 halves are still transposing. ----
            def tp_half(jc, hf):
                jcb = slice(jc * 128, (jc + 1) * 128)
                tph = ps_t.tile(
                    [128, C_LEN // 2], F16, tag="tp", name=f"tp{jc}_{hf}"
                )
                for i8 in range(8):
                    ic = hf * 8 + i8
                    nc.tensor.matmul(
                        tph[:, i8 * 128 : (i8 + 1) * 128],
                        e1_sb[:, ic, jcb],
                        idt,
                        is_transpose=True,
                    )
                hsl = slice(hf * 1024, (hf + 1) * 1024)
                # fold e^v into the transposed weights (per-partition here)
                nc.vector.tensor_scalar_mul(
                    e1t_sb[:, jc, hsl], tph, ev_sb[:, jc : jc + 1]
                )
                if BM_FP8 and hf == 0:
                    # h0 fp8 copies run mid-S0 where DVE has slack; the h1
                    # fp8 copies are issued in 512-wide pieces from the SBUF
                    # e1t16 during the first A iterations (Act)
                    nc.vector.tensor_scalar_mul(
                        e1t8_sb[:, jc, hsl], tph, ev_sb[:, jc : jc + 1]
                    )

            def t_phase(jc):
                jcb = slice(jc * 128, (jc + 1) * 128)
                t_ps = ps_mm.tile([128, DIM], F32, tag="mm")
                cs_ps = ps_vec.tile([128, 2], F32, tag="vec")
                for h in range(IC // 2):
                    lhsT = e18_sb[:, 2 * h : 2 * h + 2, jcb]
                    nc.tensor.matmul(
                        t_ps, lhsT, cn8_sb[:, 2 * h : 2 * h + 2, :],
                        start=(h == 0), stop=(h == IC // 2 - 1),
                        perf_mode=DR,
                    )
                    nc.tensor.matmul(
                        cs_ps[:, 0:1], lhsT, cf8[:, :, 0:1],
                        start=(h == 0), stop=(h == IC // 2 - 1),
                        perf_mode=DR,
                    )
                rcs = scr.tile([128, 1], F32, tag="rr")
                nc.vector.reciprocal(out=rcs, in_=cs_ps[:, 0:1])
                nc.vector.tensor_scalar_mul(tv_sb[:, jc, :], t_ps, rcs)

            u_ps = ps_vec.tile([128, IC], F32, tag="vec")
            v_qv_phase()
            s0_u_exp_block(0, 4)
            s0_u_exp_block(4, 4)
            # h0 transposes only need e1 rows ic0-7: run them mid-S0
            tp_half(0, 0)
            tp_half(1, 0)
            s0_u_exp_block(8, 4)
            tp_half(2, 0)
            tp_half(3, 0)
            s0_u_exp_block(12, 2)
            s0_u_exp_block(14, 2)
            # h1 transposes in 4-wide chunks: jc0/jc1 interleave with T,
            # jc2/jc3 spread over the first A iterations so per-ic PE work
            # stays under the output wire pace
            def tp_chunk(c):
                jcq, hh = divmod(c, 2)
                jcb = slice(jcq * 128, (jcq + 1) * 128)
                tpc = ps_t.tile([128, 512], F16, tag="tp", name=f"tpc{c}")
                for i4 in range(4):
                    nc.tensor.matmul(
                        tpc[:, i4 * 128 : (i4 + 1) * 128],
                        e1_sb[:, 8 + 4 * hh + i4, jcb],
                        idt,
                        is_transpose=True,
                    )
                seg = slice(1024 + hh * 512, 1024 + (hh + 1) * 512)
                nc.vector.tensor_scalar_mul(
                    e1t_sb[:, jcq, seg], tpc, ev_sb[:, jcq : jcq + 1]
                )
                if BM_FP8:
                    nc.scalar.mul(
                        e1t8_sb[:, jcq, seg], tpc, ev_sb[:, jcq : jcq + 1]
                    )

            t_phase(0)
            tp_chunk(0)
            tp_chunk(1)
            t_phase(1)
            tp_chunk(2)
            tp_chunk(3)
            t_phase(2)
            t_phase(3)

            # ---- A, Bm, r per ic; stage [A | C*A | C*Bm].  The h1 E1T
            # transposes are interleaved with the first ics (which only read
            # h0 columns).  Paired output DMAs, singles for the last 4. ----
            st = None
            r_tile = ps_vec.tile([128, 2, 4], F32, tag="vec")
            for ic in range(IC):
                if ic < 4:
                    tp_chunk(4 + ic)
                icb = slice(ic * 128, (ic + 1) * 128)
                a_ps = ps_mm.tile([128, DIM], F32, tag="mm")
                b_ps = ps_mm.tile([128, DIM], F32, tag="mm")
                r_ps = r_tile[:, ic % 2, :]
                for jc in range(JC):
                    lhsT = e1t_sb[:, jc, icb]
                    nc.tensor.matmul(
                        a_ps, lhsT, qp_sb[:, jc, :],
                        start=(jc == 0), stop=(jc == JC - 1),
                    )
                    nc.tensor.matmul(
                        r_ps[:, 0:2], lhsT, ones16,
                        start=(jc == 0), stop=(jc == JC - 1),
                    )
                if BM_FP8:
                    for h in range(JC // 2):
                        nc.tensor.matmul(
                            b_ps,
                            e1t8_sb[:, 2 * h : 2 * h + 2, icb],
                            tv_sb[:, 2 * h : 2 * h + 2, :],
                            start=(h == 0), stop=(h == JC // 2 - 1),
                            perf_mode=DR,
                        )
                        nc.tensor.matmul(
                            r_ps[:, 2:3],
                            e1t8_sb[:, 2 * h : 2 * h + 2, icb],
                            cf8[:, :, 0:1],
                            start=(h == 0), stop=(h == JC // 2 - 1),
                            perf_mode=DR,
                        )
                else:
                    for jc in range(JC):
                        nc.tensor.matmul(
                            b_ps, e1t_sb[:, jc, icb], tv_sb[:, jc, :],
                            start=(jc == 0), stop=(jc == JC - 1),
                        )
                rr = scr.tile([128, 1], F32, tag="rr")
                nc.vector.reciprocal(out=rr, in_=r_ps[:, 0:1])
                if BM_FP8:
                    rr8 = scr.tile([128, 1], F32, tag="rr")
                    nc.vector.reciprocal(out=rr8, in_=r_ps[:, 2:3])
                else:
                    rr8 = rr
                st = stagep.tile(
                    [128, 1, 3 * DIM], F16, tag="stage", name=f"st{ic}"
                )
                sl = st[:, 0, :]
                nc.scalar.mul(sl[:, 0:DIM], a_ps, rr)
                # gpsimd cannot touch PSUM: Pool multiplies the SBUF st_A
                # copy by C; DVE handles C*Bm straight from the psum
                nc.gpsimd.tensor_mul(
                    sl[:, DIM : 2 * DIM], sl[:, 0:DIM], cn_sb[:, ic, :]
                )
                nc.vector.scalar_tensor_tensor(
                    out=sl[:, 2 * DIM : 3 * DIM], in0=b_ps, scalar=rr8,
                    in1=cn_sb[:, ic, :], op0=MUL, op1=MUL,
                )
                if ic >= IC - 1:
                    # last blocks: ship [A|C*A] as soon as it's ready and let
                    # only the small C*Bm transfer wait on the slowest chain
                    nc.sync.dma_start(
                        out=Y[icb, 0 : 2 * DIM], in_=sl[:, 0 : 2 * DIM]
                    )
                    nc.scalar.dma_start(
                        out=Y[icb, 2 * DIM : 3 * DIM],
                        in_=sl[:, 2 * DIM : 3 * DIM],
                    )
                else:
                    nc.sync.dma_start(
                        out=Y[icb, :], in_=st.rearrange("p a e -> p (a e)")
                    )

    if split_waits:
        _split_multi_waits(nc)
    return nc


_PROGRAM = None


def _get_program():
    global _PROGRAM
    if _PROGRAM is None:
        _PROGRAM = _build_program()
    return _PROGRAM


def kernel(C, Q, C_mask, Q_mask, Wq, bq, w1, w2, w3, b):
    # Masks are all-ones and bq/b are zero for this problem (spec fills);
    # they cancel out of the computation and are not shipped to the device.
    C = np.asarray(C, np.float32)
    Q = np.asarray(Q, np.float32)
    Wq = np.asarray(Wq, np.float32)
    w1 = np.asarray(w1, np.float32)
    w2 = np.asarray(w2, np.float32)
    w3 = np.asarray(w3, np.float32)

    cf32 = np.zeros((128, KT + 1), np.float32)
    cf32[:, 0:KT] = w3.reshape(KT, 128).T
    cf32[:, KT] = -SHIFT
    cf16 = np.zeros((128, 8 + 2 + 128), np.float16)
    cf16[:, 0:KT] = w1.reshape(KT, 128).T
    cf16[:, KT : 2 * KT] = w2.reshape(KT, 128).T
    cf16[:, 8] = 1.0
    cf16[:, 10 : 10 + 128] = np.eye(128, dtype=np.float16)
    cf8 = np.zeros((128, 4), NPF8)
    cf8[:, 0] = 1.0
    cf8[:, 2] = 1.0
    wq16 = Wq.astype(np.float16)

    in_maps = []
    for bi in range(B):
        Cb = np.ascontiguousarray(C[:, bi, :])
        Cb16 = Cb.astype(np.float16)
        qw = np.concatenate(
            [Q[:, bi, :].T.astype(np.float16), wq16], axis=1
        )
        in_maps.append(
            {
                "QW": np.ascontiguousarray(qw),
                "CT": np.ascontiguousarray(Cb.T).astype(np.float16),
                "CN": Cb16,
                "CN8": Cb.astype(NPF8),
                "CF32": cf32,
                "CF16": cf16,
                "CF8": cf8,
            }
        )

    nc = _get_program()
    res = run_bass_kernel_spmd(nc, in_maps, list(range(N_CORES)))
    out = np.empty((C_LEN, B, 4 * DIM), np.float32)
    out[:, :, 0:DIM] = C  # passthrough columns assembled on host
    for c in range(N_CORES):
        out[:, c, DIM:] = res.results[c]["Y"].astype(np.float32)
    return out


# revision 76
# speedup vs baseline: 1.0014x; 1.0007x over previous
"""Coattention kernel for Trainium2, data-parallel over batch across 8 NeuronCores.

v4 design (per core, one batch).  Decomposed score S = u_i + v_j + S0_ij with
S0 = (C*w3)@Qp^T; the u/v terms are folded so that every fp8 tensor is
O(1)-ranged (a per-row scale that cancels mathematically does NOT cancel in
fp8 -- it crushes rows into subnormals):

  Qp^T    = Wq-GEMM (fp16); Qp natural via PE transpose
  E1NOU   = exp(S0 - SHIFT)            fp16, no u/v (Act, const bias)
  e18     = fp8(E1NOU * e^u)           natural [c,q]; T-path lhsT (u belongs
                                       to the column softmax)
  T path  = fp8 DoubleRow: T = e18^T @ C_8, cs = e18^T @ 1; tv = T/cs
  e1t(8)  = transpose(E1NOU) * e^v     [q,c] fp16 + fp8 (v is per-partition
                                       after transposing; row softmax weights)
  A       = (e1t^T @ Qp) / r16,  r16 = e1t^T @ [1|0]      (fp16)
  Bm      = (e1t8^T @ tv8) / r8, r8  = e1t8^T @ 1         (fp8 DoubleRow)
  out     = [A | C*A | C*Bm] staged fp16, f32 upcast + C passthrough on host

Scheduling: PE warmup ramps the p-state during the first DMAs; qt/wq ship
interleaved per k-chunk and ct in column chunks so P1/S0 chase the (single,
serialized) DMA wire; S0+exp pipeline per 4-ic block with h0 E1-transposes
mid-stream; T right after the last exp; h1 transposes and fp8 copies spread
through the first A iterations; per-ic outputs stream as singles on the SP
queue (gpsimd must never touch PSUM; all output DMAs live on sync).

Masks are all-ones and b == bq == 0 per the problem spec, so they drop out.
"""

import os
import sys

import numpy as np
import ml_dtypes

for _p in ("/opt/trn_rl_repo", "/root/.axon_site/_ro/trn_rl_repo"):
    if os.path.isdir(_p) and _p not in sys.path:
        sys.path.append(_p)

import concourse.bass as bass
import concourse.mybir as mybir
import concourse.tile as tile
from concourse.bass_utils import run_bass_kernel_spmd

C_LEN, Q_LEN, DIM, B = 2048, 512, 512, 8
N_CORES = 8
IC = C_LEN // 128   # 16 i-chunks
JC = Q_LEN // 128   # 4 j-chunks
KT = DIM // 128     # 4 k-tiles
SHIFT = 3.5         # global logit shift so fp8 E1 stays in e4m3 range

F32 = mybir.dt.float32
F16 = mybir.dt.float16
F8 = mybir.dt.float8e4
EXP = mybir.ActivationFunctionType.Exp
DR = mybir.MatmulPerfMode.DoubleRow
MUL = mybir.AluOpType.mult
NPF8 = ml_dtypes.float8_e4m3

BM_FP8 = True  # Bm matmul via fp8 DoubleRow (needs E1T8 copies)


def _split_multi_waits(nc, cap=1):
    """Walrus in this container rejects >1 sync wait per CTRL instruction;
    Tile's tail drain carries one wait per tracked processor.  Spill the
    extras onto preceding single-wait NoOps on the same engine."""
    for fn in nc.m.functions:
        for blk in fn.blocks:
            insts = list(blk.instructions)
            out, changed = [], False
            for inst in insts:
                si = inst.sync_info
                ow = si.on_wait if si is not None else None
                if ow is not None and len(ow) > cap:
                    waits = list(ow)
                    for w in waits[:-cap]:
                        nop = mybir.InstNoOp(
                            name=nc.get_next_instruction_name(), ins=[], outs=[]
                        )
                        nop.engine = inst.engine
                        nop.sync_info = mybir.SyncInfo(on_wait=[w], on_update=[])
                        out.append(nop)
                    si.on_wait = waits[-cap:]
                    changed = True
                out.append(inst)
            if changed:
                blk.instructions = out


def _build_program(split_waits=True):
    nc = bass.Bass()

    QW = nc.dram_tensor("QW", [DIM, Q_LEN + DIM], F16, kind="ExternalInput")
    CT = nc.dram_tensor("CT", [DIM, C_LEN], F16, kind="ExternalInput")
    CN = nc.dram_tensor("CN", [C_LEN, DIM], F16, kind="ExternalInput")
    CN8 = nc.dram_tensor("CN8", [C_LEN, DIM], F8, kind="ExternalInput")
    CF32 = nc.dram_tensor("CF32", [128, KT + 1], F32, kind="ExternalInput")
    CF16 = nc.dram_tensor("CF16", [128, 8 + 2 + 128], F16, kind="ExternalInput")
    CF8 = nc.dram_tensor("CF8", [128, 4], F8, kind="ExternalInput")
    Y = nc.dram_tensor("Y", [C_LEN, 3 * DIM], F16, kind="ExternalOutput")

    with tile.TileContext(nc) as tc:
        with (
            tc.tile_pool(name="consts", bufs=1) as consts,
            tc.tile_pool(name="big", bufs=1) as big,
            tc.tile_pool(name="ps_mm", bufs=5, space="PSUM") as ps_mm,
            tc.tile_pool(name="ps_t", bufs=2, space="PSUM") as ps_t,
            tc.tile_pool(name="ps_vec", bufs=1, space="PSUM") as ps_vec,
            tc.tile_pool(name="scr", bufs=3) as scr,
            tc.tile_pool(name="stage", bufs=8) as stagep,
        ):
            # ---- SBUF tiles ----
            qw_sb = big.tile([128, KT, Q_LEN + DIM], F16)  # [Q^T | Wq] per kt
            ct_sb = big.tile([128, KT, C_LEN], F16)
            cn_sb = big.tile([128, IC, DIM], F16)
            cn8_sb = big.tile([128, IC, DIM], F8)
            qpt_sb = big.tile([128, KT, Q_LEN], F16)    # Qp^T plain
            qptw3_sb = big.tile([128, KT, Q_LEN], F16)  # w3 * Qp^T
            qp_sb = big.tile([128, JC, DIM], F16)       # Qp natural
            tv_sb = big.tile([128, JC, DIM], F8 if BM_FP8 else F16)  # T/cs
            e1_sb = big.tile([128, IC, Q_LEN], F16)     # exp(S0 + u - SHIFT)
            e18_sb = big.tile([128, IC, Q_LEN], F8)     # fp8 copy
            e1t_sb = big.tile([128, JC, C_LEN], F16)    # E1 transposed
            if BM_FP8:
                e1t8_sb = big.tile([128, JC, C_LEN], F8)

            cf32 = consts.tile([128, KT + 1], F32)      # w3 columns | -SHIFT
            cf16 = consts.tile([128, 8 + 2 + 128], F16)
            cf8 = consts.tile([128, 2, 2], F8)          # [:, :, 0] = 1
            w1r = cf16[:, 0:KT]
            w2r = cf16[:, KT : 2 * KT]
            ones16 = cf16[:, 8:10]                      # [1 | 0]
            idt = cf16[:, 10 : 10 + 128]
            shiftcol = cf32[:, KT : KT + 1]
            ev_sb = consts.tile([128, JC], F32)
            eu_sb = consts.tile([128, IC], F32)

            # ---- PE warmup: junk matmuls on a memset tile ramp the p-state
            # while the first input DMAs are in flight.  They write the P1
            # psum tiles (start=True of the real P1 groups resets them, and
            # the tiles do get read, which keeps the BIR verifier happy). ----
            warm = consts.tile([128, 512], F16)
            nc.gpsimd.memset(warm, 0.5)
            qp_ps = [
                ps_mm.tile([128, Q_LEN], F32, tag="mm", name=f"qp_ps{m}")
                for m in range(KT)
            ]
            for i in range(6):
                nc.tensor.matmul(
                    qp_ps[i % KT], warm[:, 0:128], warm, start=True, stop=True
                )

            # ---- input DMAs (order = wire order; one serialized channel) ----
            for kt in range(KT):
                nc.sync.dma_start(
                    out=qw_sb[:, kt, :], in_=QW[kt * 128 : (kt + 1) * 128, :]
                )
            # ct in COLUMN chunks matching the S0 ic-blocks, so u/S0/exp for
            # each block only wait on their own chunk
            S0_BLOCKS = [(0, 4), (4, 4), (8, 4), (12, 2), (14, 2)]

            def ct_chunk(b0, bn):
                nc.sync.dma_start(
                    out=ct_sb[:, :, b0 * 128 : (b0 + bn) * 128],
                    in_=CT[:, b0 * 128 : (b0 + bn) * 128].rearrange(
                        "(k p) c -> p k c", p=128
                    ),
                )

            ct_chunk(*S0_BLOCKS[0])
            nc.sync.dma_start(out=cf16, in_=CF16[:])
            nc.sync.dma_start(out=cf32, in_=CF32[:])
            nc.sync.dma_start(
                out=cf8, in_=CF8[:].rearrange("p (a b) -> p a b", a=2)
            )
            for b0, bn in S0_BLOCKS[1:]:
                ct_chunk(b0, bn)
            for h in range(2):
                nc.sync.dma_start(
                    out=cn8_sb[:, 8 * h : 8 * h + 8, :],
                    in_=CN8[h * 1024 : (h + 1) * 1024, :].rearrange(
                        "(ic p) e -> p ic e", p=128
                    ),
                )
            for h in range(2):
                nc.sync.dma_start(
                    out=cn_sb[:, 8 * h : 8 * h + 8, :],
                    in_=CN[h * 1024 : (h + 1) * 1024, :].rearrange(
                        "(ic p) e -> p ic e", p=128
                    ),
                )

            # ---- P1: Qp^T GEMM, kt-outer to chase the qt/wq DMAs ----
            for kt in range(KT):
                for m in range(KT):
                    nc.tensor.matmul(
                        qp_ps[m],
                        qw_sb[:, kt, Q_LEN + m * 128 : Q_LEN + (m + 1) * 128],
                        qw_sb[:, kt, 0:Q_LEN],
                        start=(kt == 0),
                        stop=(kt == KT - 1),
                    )
            for m in range(KT):
                nc.scalar.copy(qpt_sb[:, m, :], qp_ps[m])
                nc.vector.tensor_scalar_mul(
                    qptw3_sb[:, m, :], qp_ps[m], cf32[:, m : m + 1]
                )

            # ---- v = Qp @ w2, ev, Qp-natural transposes, qv: deferred until
            # after S0 block 0 (they need the Act-side qpt copies; S0 only
            # needs the DVE-side qptw3, so it can start sooner) ----
            def v_qv_phase():
                v_ps = ps_vec.tile([128, JC], F32, tag="vec")
                for jc in range(JC):
                    for kt in range(KT):
                        nc.tensor.matmul(
                            v_ps[:, jc : jc + 1],
                            qpt_sb[:, kt, jc * 128 : (jc + 1) * 128],
                            w2r[:, kt : kt + 1],
                            start=(kt == 0),
                            stop=(kt == KT - 1),
                        )
                nc.scalar.activation(out=ev_sb, in_=v_ps, func=EXP)
                for jc in range(JC):
                    tpq = ps_t.tile([128, DIM], F16, tag="tp", name=f"tpq{jc}")
                    for m in range(KT):
                        nc.tensor.matmul(
                            tpq[:, m * 128 : (m + 1) * 128],
                            qpt_sb[:, m, jc * 128 : (jc + 1) * 128],
                            idt,
                            is_transpose=True,
                        )
                    nc.vector.tensor_copy(qp_sb[:, jc, :], tpq)

            # ---- S0 + u + exp pipelined per ic-block; each block only needs
            # its own ct column chunk (tapered blocks: last exps finish early)
            def s0_u_exp_block(b0, bn):
                for i4 in range(bn):
                    ic = b0 + i4
                    for kt in range(KT):
                        nc.tensor.matmul(
                            u_ps[:, ic : ic + 1],
                            ct_sb[:, kt, ic * 128 : (ic + 1) * 128],
                            w1r[:, kt : kt + 1],
                            start=(kt == 0),
                            stop=(kt == KT - 1),
                        )
                nc.scalar.activation(
                    out=eu_sb[:, b0 : b0 + bn], in_=u_ps[:, b0 : b0 + bn],
                    func=EXP,
                )
                pss = [
                    ps_mm.tile([128, Q_LEN], F32, tag="mm", name=f"s0_ps{b0}_{i}")
                    for i in range(bn)
                ]
                for i4 in range(bn):
                    ic = b0 + i4
                    for kt in range(KT):
                        nc.tensor.matmul(
                            pss[i4],
                            ct_sb[:, kt, ic * 128 : (ic + 1) * 128],
                            qptw3_sb[:, kt, :],
                            start=(kt == 0),
                            stop=(kt == KT - 1),
                        )
                for i4 in range(bn):
                    ic = b0 + i4
                    nc.scalar.activation(
                        out=e1_sb[:, ic, :], in_=pss[i4], func=EXP,
                        bias=shiftcol,
                    )
                    nc.gpsimd.tensor_scalar_mul(
                        e18_sb[:, ic, :], e1_sb[:, ic, :], eu_sb[:, ic : ic + 1]
                    )

            # ---- E1T transposes (half-tiles) interleaved with fp8-DR T/cs.
            # h0 halves (ic 0-7 columns) come first so the A-phase can start
            # while the h1 halves are still transposing. ----
            def tp_half(jc, hf):
                jcb = slice(jc * 128, (jc + 1) * 128)
                tph = ps_t.tile(
                    [128, C_LEN // 2], F16, tag="tp", name=f"tp{jc}_{hf}"
                )
                for i8 in range(8):
                    ic = hf * 8 + i8
                    nc.tensor.matmul(
                        tph[:, i8 * 128 : (i8 + 1) * 128],
                        e1_sb[:, ic, jcb],
                        idt,
                        is_transpose=True,
                    )
                hsl = slice(hf * 1024, (hf + 1) * 1024)
                # fold e^v into the transposed weights (per-partition here)
                nc.vector.tensor_scalar_mul(
                    e1t_sb[:, jc, hsl], tph, ev_sb[:, jc : jc + 1]
                )
                if BM_FP8 and hf == 0:
                    # h0 fp8 copies run mid-S0 where DVE has slack; the h1
                    # fp8 copies are issued in 512-wide pieces from the SBUF
                    # e1t16 during the first A iterations (Act)
                    nc.vector.tensor_scalar_mul(
                        e1t8_sb[:, jc, hsl], tph, ev_sb[:, jc : jc + 1]
                    )

            def t_phase(jc):
                jcb = slice(jc * 128, (jc + 1) * 128)
                t_ps = ps_mm.tile([128, DIM], F32, tag="mm")
                cs_ps = ps_vec.tile([128, 2], F32, tag="vec")
                for h in range(IC // 2):
                    lhsT = e18_sb[:, 2 * h : 2 * h + 2, jcb]
                    nc.tensor.matmul(
                        t_ps, lhsT, cn8_sb[:, 2 * h : 2 * h + 2, :],
                        start=(h == 0), stop=(h == IC // 2 - 1),
                        perf_mode=DR,
                    )
                    nc.tensor.matmul(
                        cs_ps[:, 0:1], lhsT, cf8[:, :, 0:1],
                        start=(h == 0), stop=(h == IC // 2 - 1),
                        perf_mode=DR,
                    )
                rcs = scr.tile([128, 1], F32, tag="rr")
                nc.vector.reciprocal(out=rcs, in_=cs_ps[:, 0:1])
                nc.vector.tensor_scalar_mul(tv_sb[:, jc, :], t_ps, rcs)

            u_ps = ps_vec.tile([128, IC], F32, tag="vec")
            v_qv_phase()
            s0_u_exp_block(0, 4)
            s0_u_exp_block(4, 4)
            # h0 transposes only need e1 rows ic0-7: run them mid-S0
            tp_half(0, 0)
            tp_half(1, 0)
            s0_u_exp_block(8, 4)
            tp_half(2, 0)
            tp_half(3, 0)
            s0_u_exp_block(12, 2)
            s0_u_exp_block(14, 2)
            # h1 transposes in 4-wide chunks: jc0/jc1 interleave with T,
            # jc2/jc3 spread over the first A iterations so per-ic PE work
            # stays under the output wire pace
            def tp_chunk(c):
                jcq, hh = divmod(c, 2)
                jcb = slice(jcq * 128, (jcq + 1) * 128)
                tpc = ps_t.tile([128, 512], F16, tag="tp", name=f"tpc{c}")
                for i4 in range(4):
                    nc.tensor.matmul(
                        tpc[:, i4 * 128 : (i4 + 1) * 128],
                        e1_sb[:, 8 + 4 * hh + i4, jcb],
                        idt,
                        is_transpose=True,
                    )
                seg = slice(1024 + hh * 512, 1024 + (hh + 1) * 512)
                nc.vector.tensor_scalar_mul(
                    e1t_sb[:, jcq, seg], tpc, ev_sb[:, jcq : jcq + 1]
                )
                if BM_FP8:
                    nc.scalar.mul(
                        e1t8_sb[:, jcq, seg], tpc, ev_sb[:, jcq : jcq + 1]
                    )

            t_phase(0)
            tp_chunk(0)
            tp_chunk(1)
            t_phase(1)
            tp_chunk(2)
            tp_chunk(3)
            t_phase(2)
            t_phase(3)

            # ---- A, Bm, r per ic; stage [A | C*A | C*Bm].  The h1 E1T
            # transposes are interleaved with the first ics (which only read
            # h0 columns).  Paired output DMAs, singles for the last 4. ----
            st = None
            r_tile = ps_vec.tile([128, 2, 4], F32, tag="vec")
            for ic in range(IC):
                if ic < 4:
                    tp_chunk(4 + ic)
                icb = slice(ic * 128, (ic + 1) * 128)
                a_ps = ps_mm.tile([128, DIM], F32, tag="mm")
                b_ps = ps_mm.tile([128, DIM], F32, tag="mm")
                r_ps = r_tile[:, ic % 2, :]
                for jc in range(JC):
                    lhsT = e1t_sb[:, jc, icb]
                    nc.tensor.matmul(
                        a_ps, lhsT, qp_sb[:, jc, :],
                        start=(jc == 0), stop=(jc == JC - 1),
                    )
                    nc.tensor.matmul(
                        r_ps[:, 0:2], lhsT, ones16,
                        start=(jc == 0), stop=(jc == JC - 1),
                    )
                if BM_FP8:
                    for h in range(JC // 2):
                        nc.tensor.matmul(
                            b_ps,
                            e1t8_sb[:, 2 * h : 2 * h + 2, icb],
                            tv_sb[:, 2 * h : 2 * h + 2, :],
                            start=(h == 0), stop=(h == JC // 2 - 1),
                            perf_mode=DR,
                        )
                        nc.tensor.matmul(
                            r_ps[:, 2:3],
                            e1t8_sb[:, 2 * h : 2 * h + 2, icb],
                            cf8[:, :, 0:1],
                            start=(h == 0), stop=(h == JC // 2 - 1),
                            perf_mode=DR,
                        )
                else:
                    for jc in range(JC):
                        nc.tensor.matmul(
                            b_ps, e1t_sb[:, jc, icb], tv_sb[:, jc, :],
                            start=(jc == 0), stop=(jc == JC - 1),
                        )
                rr = scr.tile([128, 1], F32, tag="rr")
                nc.vector.reciprocal(out=rr, in_=r_ps[:, 0:1])
                if BM_FP8:
                    rr8 = scr.tile([128, 1], F32, tag="rr")
                    nc.vector.reciprocal(out=rr8, in_=r_ps[:, 2:3])
                else:
                    rr8 = rr
                st = stagep.tile(
                    [128, 1, 3 * DIM], F16, tag="stage", name=f"st{ic}"
                )
                sl = st[:, 0, :]
                nc.scalar.mul(sl[:, 0:DIM], a_ps, rr)
                # gpsimd cannot touch PSUM: Pool multiplies the SBUF st_A
                # copy by C; DVE handles C*Bm straight from the psum
                nc.gpsimd.tensor_mul(
                    sl[:, DIM : 2 * DIM], sl[:, 0:DIM], cn_sb[:, ic, :]
                )
                nc.vector.scalar_tensor_tensor(
                    out=sl[:, 2 * DIM : 3 * DIM], in0=b_ps, scalar=rr8,
                    in1=cn_sb[:, ic, :], op0=MUL, op1=MUL,
                )
                if ic >= IC - 3:
                    # last blocks: ship [A|C*A] as soon as it's ready and let
                    # only the small C*Bm transfer wait on the slowest chain
                    nc.sync.dma_start(
                        out=Y[icb, 0 : 2 * DIM], in_=sl[:, 0 : 2 * DIM]
                    )
                    nc.scalar.dma_start(
                        out=Y[icb, 2 * DIM : 3 * DIM],
                        in_=sl[:, 2 * DIM : 3 * DIM],
                    )
                else:
                    nc.sync.dma_start(
                        out=Y[icb, :], in_=st.rearrange("p a e -> p (a e)")
                    )

    if split_waits:
        _split_multi_waits(nc)
    return nc


_PROGRAM = None


def _get_program():
    global _PROGRAM
    if _PROGRAM is None:
        _PROGRAM = _build_program()
    return _PROGRAM


def kernel(C, Q, C_mask, Q_mask, Wq, bq, w1, w2, w3, b):
    # Masks are all-ones and bq/b are zero for this problem (spec fills);
    # they cancel out of the computation and are not shipped to the device.
    C = np.asarray(C, np.float32)
    Q = np.asarray(Q, np.float32)
    Wq = np.asarray(Wq, np.float32)
    w1 = np.asarray(w1, np.float32)
    w2 = np.asarray(w2, np.float32)
    w3 = np.asarray(w3, np.float32)

    cf32 = np.zeros((128, KT + 1), np.float32)
    cf32[:, 0:KT] = w3.reshape(KT, 128).T
    cf32[:, KT] = -SHIFT
    cf16 = np.zeros((128, 8 + 2 + 128), np.float16)
    cf16[:, 0:KT] = w1.reshape(KT, 128).T
    cf16[:, KT : 2 * KT] = w2.reshape(KT, 128).T
    cf16[:, 8] = 1.0
    cf16[:, 10 : 10 + 128] = np.eye(128, dtype=np.float16)
    cf8 = np.zeros((128, 4), NPF8)
    cf8[:, 0] = 1.0
    cf8[:, 2] = 1.0
    wq16 = Wq.astype(np.float16)

    in_maps = []
    for bi in range(B):
        Cb = np.ascontiguousarray(C[:, bi, :])
        Cb16 = Cb.astype(np.float16)
        qw = np.concatenate(
            [Q[:, bi, :].T.astype(np.float16), wq16], axis=1
        )
        in_maps.append(
            {
                "QW": np.ascontiguousarray(qw),
                "CT": np.ascontiguousarray(Cb.T).astype(np.float16),
                "CN": Cb16,
                "CN8": Cb.astype(NPF8),
                "CF32": cf32,
                "CF16": cf16,
                "CF8": cf8,
            }
        )

    nc = _get_program()
    res = run_bass_kernel_spmd(nc, in_maps, list(range(N_CORES)))
    out = np.empty((C_LEN, B, 4 * DIM), np.float32)
    out[:, :, 0:DIM] = C  # passthrough columns assembled on host
    for c in range(N_CORES):
        out[:, c, DIM:] = res.results[c]["Y"].astype(np.float32)
    return out
